# revision 2
# baseline (speedup 1.0000x reference)
"""GP prediction kernel for Trainium2 (8 NeuronCores, data-parallel over batch).

Computes z_pred[b, p, d] = sum_c k_mult[p, c] * z_enc[b, c, d] where k_mult
is the [64, 448] GP weight matrix k_pred.T @ inv(cov + sigma*I). k_mult
depends only on compile-time constants, so it is precomputed on host; the
device work is a batched [64,448] @ [448,1024] matmul, sharded 8 batches
per core.

Both operands and the output travel as fp16 with fp32 PSUM accumulation.
The correctness gate is rel_err < 2e-2; fp16 rounding of z/k/out contributes
~1e-3, so a single fp16 term suffices — this halves HBM traffic and cuts
tensor-engine work 3x vs an fp32-equivalent hi/lo split.
"""
import numpy as np
from contextlib import ExitStack

import concourse.bacc as bacc
import concourse.tile as tile
from concourse import mybir
from concourse.bass_utils import run_bass_kernel_spmd

# Problem constants (hardcoded per harness contract).
B, T, D = 64, 512, 1024
P = 64                 # N_PREDICTORS
C = T - P              # 448 context timesteps
L, SIGMA, TIMESCALE = 0.01, 0.01, 0.3
N_CORES = 8
BPC = B // N_CORES     # batches per core

KJ = [128, 128, 128, 64]          # K-tile sizes along the contraction dim
KOFF = [0, 128, 256, 384]


def _k_mult_T() -> np.ndarray:
    """[C, P] transpose of the GP weight matrix, solved in float64 on host."""
    t = np.linspace(0.0, 1.0, T)
    t_in = t[:C] * TIMESCALE
    t_pred = t[C:] * TIMESCALE

    def rbf_np(x, y):
        d = x[:, None] - y[None, :]
        return np.exp(-0.5 * d * d / L)

    cov = rbf_np(t_in, t_in) + np.eye(C) * SIGMA
    km_T = np.linalg.solve(cov, rbf_np(t_in, t_pred))   # [C, P]
    return np.ascontiguousarray(km_T.astype(np.float16))


KM_T = _k_mult_T()

_NC = None


def _build():
    nc = bacc.Bacc()
    z = nc.dram_tensor("z", [BPC * C, D], mybir.dt.float16, kind="ExternalInput")
    km = nc.dram_tensor("km", [C, P], mybir.dt.float16, kind="ExternalInput")
    out = nc.dram_tensor("out", [BPC * P, D], mybir.dt.float16, kind="ExternalOutput")

    with tile.TileContext(nc) as tc, ExitStack() as ctx:
        kpool = ctx.enter_context(tc.tile_pool(name="km", bufs=1))
        zpool = ctx.enter_context(tc.tile_pool(name="z", bufs=16))
        opool = ctx.enter_context(tc.tile_pool(name="o", bufs=3))
        ppool = ctx.enter_context(tc.tile_pool(name="ps", bufs=8, space="PSUM"))

        # k_mult.T staged once: col block j = K-tile j of the [C, P] matrix
        km_sb = kpool.tile([128, 4 * P], mybir.dt.float16)
        for j in range(4):
            nc.sync.dma_start(
                km_sb[: KJ[j], j * P : (j + 1) * P],
                km[KOFF[j] : KOFF[j] + KJ[j], :],
            )

        def km_j(j):
            return km_sb[: KJ[j], j * P : (j + 1) * P]

        for bp in range(BPC // 2):  # batch pairs -> [128, D] output tiles
            out_sb = opool.tile([128, D], mybir.dt.float16)
            for half in range(2):
                b = 2 * bp + half
                zt = [
                    zpool.tile([128, D], mybir.dt.float16,
                               name=f"zt{b}_{j}", tag="zt")
                    for j in range(4)
                ]
                # split z loads across both HWDGE queues
                for j in range(4):
                    eng = nc.sync if j % 2 == 0 else nc.scalar
                    eng.dma_start(
                        zt[j][: KJ[j], :],
                        z[b * C + KOFF[j] : b * C + KOFF[j] + KJ[j], :],
                    )

                for n in range(2):  # 512-wide PSUM column halves
                    ps = ppool.tile([P, 512], mybir.dt.float32)
                    for j in range(4):
                        nc.tensor.matmul(
                            ps[:, :], km_j(j),
                            zt[j][: KJ[j], n * 512 : (n + 1) * 512],
                            start=(j == 0), stop=(j == 3),
                        )
                    nc.vector.tensor_copy(
                        out_sb[half * P : (half + 1) * P, n * 512 : (n + 1) * 512],
                        ps[:, :],
                    )
            nc.sync.dma_start(out[bp * 128 : (bp + 1) * 128, :], out_sb[:])

    nc.finalize()
    return nc


def kernel(z_enc: np.ndarray, _trace: bool = False):
    global _NC
    z_enc = np.asarray(z_enc, dtype=np.float32)
    if _NC is None:
        _NC = _build()

    z16 = z_enc[:, :C, :].astype(np.float16)
    in_maps = [
        {"z": z16[i * BPC : (i + 1) * BPC].reshape(BPC * C, D), "km": KM_T}
        for i in range(N_CORES)
    ]

    res = run_bass_kernel_spmd(_NC, in_maps, core_ids=list(range(N_CORES)),
                               trace=_trace)
    out = np.concatenate(
        [r["out"].reshape(BPC, P, D) for r in res.results], axis=0
    ).astype(np.float32)
    if _trace:
        return out, res
    return out


# revision 3
# speedup vs baseline: 1.0106x; 1.0106x over previous
"""GP prediction kernel for Trainium2 (8 NeuronCores, data-parallel over batch).

Computes z_pred[b, p, d] = sum_c k_mult[p, c] * z_enc[b, c, d] where k_mult
is the [64, 448] GP weight matrix k_pred.T @ inv(cov + sigma*I). k_mult
depends only on compile-time constants, so it is precomputed on host; the
device work is a batched [64,448] @ [448,1024] matmul, sharded 8 batches
per core.

Everything travels as fp16 with fp32 PSUM accumulation (gate is 2e-2;
fp16 rounding contributes ~1e-3). z is repacked on host so that each SBUF
partition's data is one contiguous DRAM run: per batch a [128, 3*1024]
tile holds contraction rows c = j*128 + p for K-tiles j=0..2, and per
batch-pair a [64, 2*1024] tile holds the j=3 rows (c=384..447) of both
batches side by side. This turns the DMA packets from 2KB row-gathers
into 4-6KB linear runs, which is what the per-queue DMA bandwidth wants.
DMAs alternate between the two HWDGE queues (SP + Activation) to use
both at once.
"""
import numpy as np
from contextlib import ExitStack

import concourse.bacc as bacc
import concourse.tile as tile
from concourse import mybir
from concourse.bass_utils import run_bass_kernel_spmd

# Problem constants (hardcoded per harness contract).
B, T, D = 64, 512, 1024
P = 64                 # N_PREDICTORS
C = T - P              # 448 context timesteps
L, SIGMA, TIMESCALE = 0.01, 0.01, 0.3
N_CORES = 8
BPC = B // N_CORES     # batches per core


def _k_mult_T() -> np.ndarray:
    """[C, P] transpose of the GP weight matrix, solved in float64 on host."""
    t = np.linspace(0.0, 1.0, T)
    t_in = t[:C] * TIMESCALE
    t_pred = t[C:] * TIMESCALE

    def rbf_np(x, y):
        d = x[:, None] - y[None, :]
        return np.exp(-0.5 * d * d / L)

    cov = rbf_np(t_in, t_in) + np.eye(C) * SIGMA
    return np.linalg.solve(cov, rbf_np(t_in, t_pred))   # [C, P] float64


def _km_dev() -> np.ndarray:
    """[128, 4*P] fp16: col block j<3 = km_T rows j*128+p; block 3 (p<64) =
    km_T rows 384+p."""
    km_T = _k_mult_T().astype(np.float16)
    dev = np.zeros((128, 4 * P), np.float16)
    for j in range(3):
        dev[:, j * P : (j + 1) * P] = km_T[j * 128 : (j + 1) * 128]
    dev[:64, 3 * P : 4 * P] = km_T[384:448]
    return dev


KM_DEV = _km_dev()

_NC = None


def _build():
    nc = bacc.Bacc()
    # per batch: [128, 3*1024] rows c = j*128+p, col j*1024+d
    z = nc.dram_tensor("z", [BPC * 128, 3 * D], mybir.dt.float16,
                       kind="ExternalInput")
    # per batch pair: [64, 2*1024] rows c = 384+p, col half*1024+d
    z3 = nc.dram_tensor("z3", [(BPC // 2) * 64, 2 * D], mybir.dt.float16,
                        kind="ExternalInput")
    km = nc.dram_tensor("km", [128, 4 * P], mybir.dt.float16,
                        kind="ExternalInput")
    out = nc.dram_tensor("out", [BPC * P, D], mybir.dt.float16,
                         kind="ExternalOutput")

    with tile.TileContext(nc) as tc, ExitStack() as ctx:
        kpool = ctx.enter_context(tc.tile_pool(name="km", bufs=1))
        zpool = ctx.enter_context(tc.tile_pool(name="z", bufs=5))
        z3pool = ctx.enter_context(tc.tile_pool(name="z3", bufs=3))
        opool = ctx.enter_context(tc.tile_pool(name="o", bufs=4))
        ppool = ctx.enter_context(tc.tile_pool(name="ps", bufs=8, space="PSUM"))

        km_sb = kpool.tile([128, 4 * P], mybir.dt.float16)
        nc.scalar.dma_start(km_sb[:, :], km[:, :])

        for bp in range(BPC // 2):
            z3t = z3pool.tile([64, 2 * D], mybir.dt.float16, name=f"z3_{bp}",
                              tag="z3")
            eng3 = nc.sync if bp % 2 == 0 else nc.scalar
            eng3.dma_start(z3t[:, :], z3[bp * 64 : (bp + 1) * 64, :])

            for half in range(2):
                b = 2 * bp + half
                zb = zpool.tile([128, 3 * D], mybir.dt.float16, name=f"z_{b}",
                                tag="z")
                engz = nc.sync if b % 2 == 0 else nc.scalar
                engz.dma_start(zb[:, :], z[b * 128 : (b + 1) * 128, :])

                out_sb = opool.tile([P, D], mybir.dt.float16, name=f"o_{b}",
                                    tag="o")
                for n in range(2):
                    ps = ppool.tile([P, 512], mybir.dt.float32)
                    for j in range(3):
                        nc.tensor.matmul(
                            ps[:, :],
                            km_sb[:, j * P : (j + 1) * P],
                            zb[:, j * D + n * 512 : j * D + (n + 1) * 512],
                            start=(j == 0), stop=False,
                        )
                    nc.tensor.matmul(
                        ps[:, :],
                        km_sb[:64, 3 * P : 4 * P],
                        z3t[:, half * D + n * 512 : half * D + (n + 1) * 512],
                        start=False, stop=True,
                    )
                    nc.vector.tensor_copy(
                        out_sb[:, n * 512 : (n + 1) * 512], ps[:, :]
                    )
                engo = nc.sync if b % 2 == 1 else nc.scalar
                engo.dma_start(out[b * P : (b + 1) * P, :], out_sb[:, :])

    nc.finalize()
    return nc


def kernel(z_enc: np.ndarray, _trace: bool = False):
    global _NC
    z_enc = np.asarray(z_enc, dtype=np.float32)
    if _NC is None:
        _NC = _build()

    z16 = z_enc[:, :C, :].astype(np.float16)          # [B, 448, 1024]
    # [B, 384, D] -> (b, p, j, d) so each partition row is 3*2KB contiguous
    zmain = np.ascontiguousarray(
        z16[:, :384, :].reshape(B, 3, 128, D).transpose(0, 2, 1, 3)
    ).reshape(B, 128, 3 * D)
    # [B, 64, D] tail rows -> pair layout [64, 2*D]
    ztail = np.ascontiguousarray(
        z16[:, 384:, :].reshape(B // 2, 2, 64, D).transpose(0, 2, 1, 3)
    ).reshape(B // 2, 64, 2 * D)

    in_maps = [
        {
            "z": zmain[i * BPC : (i + 1) * BPC].reshape(BPC * 128, 3 * D),
            "z3": ztail[i * (BPC // 2) : (i + 1) * (BPC // 2)].reshape(
                (BPC // 2) * 64, 2 * D),
            "km": KM_DEV,
        }
        for i in range(N_CORES)
    ]

    res = run_bass_kernel_spmd(_NC, in_maps, core_ids=list(range(N_CORES)),
                               trace=_trace)
    out = np.concatenate(
        [r["out"].reshape(BPC, P, D) for r in res.results], axis=0
    ).astype(np.float32)
    if _trace:
        return out, res
    return out
